# revision 14
# baseline (speedup 1.0000x reference)
"""Trainium2 Bass kernel for the ESN (echo state network) forward scan.

  x_{t+1} = (1-a) x_t + a tanh(u_t + x_t @ W),  a = 0.5
  U = einsum('bit,in->tbn', Input, W_in);  out X[b,n,t] = x_{t+1}[b,n]

Sharding: data-parallel over batch (B=64 -> 8 cores x 8 batches).
W, W_in replicated; no collectives. Each core runs the full T=2000 scan
for its 8 batches and writes its [8, 1024, 2000] output slice.

Per-core per-step data flow (all on-chip, only X streamed out):
  PE:  z[8,1024](PSUM) = xT.T @ W (8 k-tile matmuls x 2 psum banks)
                        + inp_t.T @ W_in (u folded in as a K=16 matmul)
  DVE: z -> zT [128, 64] (32x32 block transposes, strided APs)
  ACT: hT = tanh(zT)
  DVE: s = xT + hT; xT' = 0.5 s; obuf[:, :, t] = 0.5 s   (x_{t+1})
Output chunks of Tc steps buffered in SBUF, DMA'd as [128, Tc]-contiguous
blocks into X[b, 128g:128g+128, t0:t0+Tc].
"""

import copy
import math
import os
import numpy as np

import concourse.bass as bass
import concourse.mybir as mybir
import concourse.tile as tile
from concourse.bass import ds
from concourse.bass_utils import run_bass_kernel_spmd

FP32 = mybir.dt.float32
FP16 = mybir.dt.float16

ALPHA = 0.5
N_CORES = 8
B, N_IN, T, N = 64, 16, 2000, 1024
TC = 100  # steps buffered per output chunk

LAST_EXEC_NS = None
_CACHED_NC = None


def _split_excess_waits(nc, limit=1):
    """The walrus build in this container rejects instructions carrying more
    than one sem wait; hoist extra waits onto same-engine NoOps."""
    import bass_rust
    for f in nc.m.functions:
        for bb in f.blocks:
            new_insts = []
            for ins in bb.instructions:
                si = ins.sync_info
                if si is not None and si.on_wait and len(si.on_wait) > limit:
                    waits = list(si.on_wait)
                    head, tail = waits[:-limit], waits[-limit:]
                    for j, w in enumerate(head):
                        c = bass_rust.InstNoOp(name=f"{ins.name}-w{j}")
                        c.engine = ins.engine
                        c.sync_info = mybir.SyncInfo(on_wait=[w], on_update=[])
                        new_insts.append(c)
                    si.on_wait = tail
                new_insts.append(ins)
            bb.instructions = new_insts
    return nc


def _build_nc(n=N, t_total=T, tc_steps=TC, n_in=N_IN, bc=B // N_CORES):
    G = n // 128
    n_chunks = t_total // tc_steps
    NB = (n + 511) // 512
    nb_sizes = [min(512, n - 512 * i) for i in range(NB)]

    assert NB == 2 and G % 2 == 0
    Gh = G // 2  # g-tiles per n-half

    nc = bass.Bass()
    w_dram = nc.dram_tensor("w", [128, G * n], FP16, kind="ExternalInput")
    win_dram = nc.dram_tensor("win", [n_in, n], FP16, kind="ExternalInput")
    inpT_dram = nc.dram_tensor("inpT", [n_in, t_total, bc], FP16,
                               kind="ExternalInput")
    x_dram = nc.dram_tensor("xout", [bc, n, t_total], FP32,
                            kind="ExternalOutput")
    x_dram_r = x_dram.rearrange("b (g p) t -> p g b t", p=128)

    with tile.TileContext(nc) as tc:
        with (
            tc.tile_pool(name="const", bufs=1) as const_pool,
            tc.tile_pool(name="state", bufs=1) as state_pool,
            tc.tile_pool(name="work", bufs=3) as work_pool,
            tc.tile_pool(name="obuf", bufs=2) as obuf_pool,
            tc.tile_pool(name="inp", bufs=2) as inp_pool,
            tc.tile_pool(name="psum", bufs=2, space="PSUM") as psum_pool,
        ):
            w_sb = const_pool.tile([128, G * n], FP16)
            nc.sync.dma_start(w_sb[:, :], w_dram[:, :])
            win_sb = const_pool.tile([n_in, n], FP16)
            nc.sync.dma_start(win_sb[:, :], win_dram[:, :])

            # State kept three ways, split per n-half for fine dep granularity:
            #   s16[.]  fp16 unscaled sum s_t = x_t + h_t -> matmul operand
            #           (the 0.5 leak is folded into W host-side)
            #   xT[.]   fp32 master of x_{t+1} = 0.5 s_t (exact output)
            #   xh16[.] fp16 of x_t, feeds the critical add s16 = xh16 + h
            s16s = [[state_pool.tile([128, Gh * 32], FP16, name=f"s16_{b_}_{h_}")
                     for h_ in range(2)] for b_ in range(2)]
            xTs = [[state_pool.tile([128, Gh * 32], FP32, name=f"xT{b_}_{h_}")
                    for h_ in range(2)] for b_ in range(2)]
            xh16s = [[state_pool.tile([128, Gh * 32], FP16,
                                      name=f"xh16_{b_}_{h_}")
                      for h_ in range(2)] for b_ in range(2)]
            for b_ in range(2):
                for h_ in range(2):
                    nc.vector.memset(s16s[b_][h_][:, :], 0.0)
                    nc.vector.memset(xTs[b_][h_][:, :], 0.0)
                    nc.vector.memset(xh16s[b_][h_][:, :], 0.0)

            def chunk_body(ci):
                inp_sb = inp_pool.tile([n_in, tc_steps * bc], FP16)
                nc.sync.dma_start(
                    inp_sb[:, :], inpT_dram[:, ds(ci * tc_steps, tc_steps), :])
                obuf = obuf_pool.tile([128, G * 8 * tc_steps], FP32)
                obuf_r = obuf[:, :].rearrange(
                    "p (g b t) -> p g b t", g=G, b=8, t=tc_steps)

                def alloc_z_and_u(t):
                    # u for step t lands in fresh psum banks ahead of time;
                    # z-matmuls later accumulate on top (u sets has_written)
                    zp = [psum_pool.tile([32, 512], FP32, tag=f"z{h}",
                                         name=f"zps{h}")
                          for h in range(2)]
                    for h in range(2):
                        nc.tensor.matmul(
                            zp[h][0:8, :],
                            inp_sb[:, t * bc: (t + 1) * bc],
                            win_sb[:, 512 * h: 512 * (h + 1)],
                            start=True, stop=False,
                        )
                    return zp

                zps_cur = alloc_z_and_u(0)
                for t in range(tc_steps):
                    xT, xT_n = xTs[t % 2], xTs[(t + 1) % 2]
                    xh16, xh16_n = xh16s[t % 2], xh16s[(t + 1) % 2]
                    s16, s16_n = s16s[t % 2], s16s[(t + 1) % 2]
                    zps = zps_cur
                    for h in range(2):
                        nsl = slice(512 * h, 512 * (h + 1))
                        for g in range(G):
                            nc.tensor.matmul(
                                zps[h][0:8, :],
                                s16[g // Gh][:, (g % Gh) * 32: (g % Gh) * 32 + 8],
                                w_sb[:, g * n + nsl.start: g * n + nsl.stop],
                                start=False, stop=(g == G - 1),
                            )
                    if t + 1 < tc_steps:
                        zps_cur = alloc_z_and_u(t + 1)  # fills the PE tail gap
                    for h in range(2):
                        # z half -> zT [128, Gh*32]
                        zT = work_pool.tile([128, Gh * 32], FP32, tag=f"zT{h}",
                                            name=f"zT{h}")
                        hT = work_pool.tile([128, Gh * 32], FP32, tag=f"hT{h}",
                                            name=f"hT{h}")
                        s = work_pool.tile([128, Gh * 32], FP32, tag=f"s{h}",
                                           name=f"s{h}")
                        s_r = s[:, :].rearrange("p (g b) -> p g b", g=Gh, b=32)
                        z_r = zps[h][0:32, :].rearrange(
                            "p (g r q) -> p r g q", g=Gh, r=4, q=32)
                        for r in range(4):
                            nc.vector.transpose(
                                zT[32 * r: 32 * r + 32, :].rearrange(
                                    "p (g q) -> p g q", q=32),
                                z_r[:, r, :, :],
                            )
                        nc.scalar.activation(
                            hT[:, :], zT[:, :],
                            mybir.ActivationFunctionType.Tanh)
                        # critical: next matmul operand in one add
                        nc.vector.tensor_add(
                            s16_n[h][:, :], xh16[h][:, :], hT[:, :])
                        # off the critical path: fp32 master + output
                        nc.vector.tensor_add(s[:, :], xT[h][:, :], hT[:, :])
                        nc.scalar.mul(xT_n[h][:, :], s[:, :], ALPHA)
                        nc.vector.tensor_scalar_mul(
                            xh16_n[h][:, :], s[:, :], ALPHA)
                        nc.scalar.mul(
                            obuf_r[:, Gh * h: Gh * (h + 1), :, t],
                            s_r[:, :, 0:8], ALPHA)

                for g in range(G):
                    nc.sync.dma_start(
                        x_dram_r[:, g, :, ds(ci * tc_steps, tc_steps)],
                        obuf_r[:, g, :, :],
                    )

            with tc.For_i(0, n_chunks, 1) as i:
                chunk_body(i)

    _split_excess_waits(nc)
    return nc


def kernel(Input, W_in, W):
    """Full inputs in, full output out. Shards batch over 8 NeuronCores."""
    global LAST_EXEC_NS, _CACHED_NC
    Input = np.ascontiguousarray(np.asarray(Input, dtype=np.float32))
    W_in = np.ascontiguousarray(np.asarray(W_in, dtype=np.float32))
    W = np.ascontiguousarray(np.asarray(W, dtype=np.float32))
    Bf, n_in, t_total = Input.shape
    n = W.shape[0]
    G = n // 128
    bc = Bf // N_CORES

    if _CACHED_NC is None:
        _CACHED_NC = _build_nc(n=n, t_total=t_total, n_in=n_in, bc=bc)
    nc = _CACHED_NC

    # leak factor folded into W: matmul operand is s = x + h = 2x, so W/2
    w_r = np.ascontiguousarray(
        (ALPHA * W).reshape(G, 128, n).transpose(1, 0, 2).reshape(128, G * n)
    ).astype(np.float16)
    win16 = W_in.astype(np.float16)
    in_maps = []
    for c in range(N_CORES):
        inpT = np.ascontiguousarray(
            Input[c * bc:(c + 1) * bc].transpose(1, 2, 0)).astype(np.float16)
        in_maps.append({"w": w_r, "win": win16, "inpT": inpT})

    trace = bool(int(os.environ.get("ESN_TRACE", "0")))
    res = run_bass_kernel_spmd(
        nc, in_maps, core_ids=list(range(N_CORES)), trace=trace)
    LAST_EXEC_NS = res.exec_time_ns

    out = np.concatenate([res.results[c]["xout"] for c in range(N_CORES)],
                         axis=0)
    return np.ascontiguousarray(out.astype(np.float32))


# revision 18
# speedup vs baseline: 1.0863x; 1.0863x over previous
"""Trainium2 Bass kernel for the ESN (echo state network) forward scan.

  x_{t+1} = (1-a) x_t + a tanh(u_t + x_t @ W),  a = 0.5
  U = einsum('bit,in->tbn', Input, W_in);  out X[b,n,t] = x_{t+1}[b,n]

Sharding: data-parallel over batch (B=64 -> 8 cores x 8 batches).
W, W_in replicated; no collectives. Each core runs the full T=2000 scan
for its 8 batches and writes its [8, 1024, 2000] output slice.

Per-core per-step data flow (all on-chip, only X streamed out):
  PE:  z[8,1024](PSUM) = xT.T @ W (8 k-tile matmuls x 2 psum banks)
                        + inp_t.T @ W_in (u folded in as a K=16 matmul)
  DVE: z -> zT [128, 64] (32x32 block transposes, strided APs)
  ACT: hT = tanh(zT)
  DVE: s = xT + hT; xT' = 0.5 s; obuf[:, :, t] = 0.5 s   (x_{t+1})
Output chunks of Tc steps buffered in SBUF, DMA'd as [128, Tc]-contiguous
blocks into X[b, 128g:128g+128, t0:t0+Tc].
"""

import copy
import math
import os
import numpy as np

import concourse.bass as bass
import concourse.mybir as mybir
import concourse.tile as tile
from concourse.bass import ds
from concourse.bass_utils import run_bass_kernel_spmd

FP32 = mybir.dt.float32
FP16 = mybir.dt.float16

ALPHA = 0.5
N_CORES = 8
B, N_IN, T, N = 64, 16, 2000, 1024
TC = 100  # steps buffered per output chunk

LAST_EXEC_NS = None
_CACHED_NC = None


def _split_excess_waits(nc, limit=1):
    """The walrus build in this container rejects instructions carrying more
    than one sem wait; hoist extra waits onto same-engine NoOps."""
    import bass_rust
    for f in nc.m.functions:
        for bb in f.blocks:
            new_insts = []
            for ins in bb.instructions:
                si = ins.sync_info
                if si is not None and si.on_wait and len(si.on_wait) > limit:
                    waits = list(si.on_wait)
                    head, tail = waits[:-limit], waits[-limit:]
                    for j, w in enumerate(head):
                        c = bass_rust.InstNoOp(name=f"{ins.name}-w{j}")
                        c.engine = ins.engine
                        c.sync_info = mybir.SyncInfo(on_wait=[w], on_update=[])
                        new_insts.append(c)
                    si.on_wait = tail
                new_insts.append(ins)
            bb.instructions = new_insts
    return nc


def _build_nc(n=N, t_total=T, tc_steps=TC, n_in=N_IN, bc=B // N_CORES):
    G = n // 128
    n_chunks = t_total // tc_steps
    NB = (n + 511) // 512
    nb_sizes = [min(512, n - 512 * i) for i in range(NB)]

    assert NB == 2 and G % 2 == 0
    Gh = G // 2  # g-tiles per n-half

    nc = bass.Bass()
    sel_dram = nc.dram_tensor("sel", [128, 8], FP16, kind="ExternalInput")
    w_dram = nc.dram_tensor("w", [128, G * n], FP16, kind="ExternalInput")
    win_dram = nc.dram_tensor("win", [n_in, n], FP16, kind="ExternalInput")
    inpT_dram = nc.dram_tensor("inpT", [n_in, t_total, bc], FP16,
                               kind="ExternalInput")
    x_dram = nc.dram_tensor("xout", [bc, n, t_total], FP32,
                            kind="ExternalOutput")
    x_dram_r = x_dram.rearrange("b (g p) t -> p g b t", p=128)

    with tile.TileContext(nc) as tc:
        with (
            tc.tile_pool(name="const", bufs=1) as const_pool,
            tc.tile_pool(name="state", bufs=1) as state_pool,
            tc.tile_pool(name="work", bufs=3) as work_pool,
            tc.tile_pool(name="obuf", bufs=2) as obuf_pool,
            tc.tile_pool(name="inp", bufs=2) as inp_pool,
            tc.tile_pool(name="psum", bufs=2, space="PSUM") as psum_pool,
            tc.tile_pool(name="psumS", bufs=1, space="PSUM") as psum_static,
        ):
            w_sb = const_pool.tile([128, G * n], FP16)
            nc.sync.dma_start(w_sb[:, :], w_dram[:, :])
            win_sb = const_pool.tile([n_in, n], FP16)
            nc.sync.dma_start(win_sb[:, :], win_dram[:, :])
            sel_sb = const_pool.tile([128, 8], FP16)
            nc.sync.dma_start(sel_sb[:, :], sel_dram[:, :])
            zero16 = const_pool.tile([128, 512], FP16)
            nc.vector.memset(zero16[:, :], 0.0)

            # 4 static psum banks for the col-tiled z partials (ping-pong per
            # step); zero-filled once so never-written partition rows stay
            # finite (sel rows are 0 there, and PE treats 0*garbage as NaN
            # if garbage were NaN)
            zpsS = [[psum_static.tile([128, 512], FP32, name=f"zps_{h_}_{b_}")
                     for b_ in range(2)] for h_ in range(2)]
            for h_ in range(2):
                for b_ in range(2):
                    nc.tensor.matmul(
                        zpsS[h_][b_][:, :], zero16[:, 0:128], zero16[:, :],
                        start=True, stop=True, skip_group_check=True)

            # State kept three ways, split per n-half for fine dep granularity:
            #   s16[.]  fp16 unscaled sum s_t = x_t + h_t -> matmul operand
            #           (the 0.5 leak is folded into W host-side)
            #   xT[.]   fp32 master of x_{t+1} = 0.5 s_t (exact output)
            #   xh16[.] fp16 of x_t, feeds the critical add s16 = xh16 + h
            s16s = [[state_pool.tile([128, Gh * 8], FP16, name=f"s16_{b_}_{h_}")
                     for h_ in range(2)] for b_ in range(2)]
            xTs = [[state_pool.tile([128, Gh * 8], FP32, name=f"xT{b_}_{h_}")
                    for h_ in range(2)] for b_ in range(2)]
            xh16s = [[state_pool.tile([128, Gh * 8], FP16,
                                      name=f"xh16_{b_}_{h_}")
                      for h_ in range(2)] for b_ in range(2)]
            for b_ in range(2):
                for h_ in range(2):
                    nc.vector.memset(s16s[b_][h_][:, :], 0.0)
                    nc.vector.memset(xTs[b_][h_][:, :], 0.0)
                    nc.vector.memset(xh16s[b_][h_][:, :], 0.0)

            def chunk_body(ci):
                inp_sb = inp_pool.tile([n_in, tc_steps * bc], FP16)
                nc.sync.dma_start(
                    inp_sb[:, :], inpT_dram[:, ds(ci * tc_steps, tc_steps), :])
                obuf = obuf_pool.tile([128, G * 8 * tc_steps], FP32)
                obuf_r = obuf[:, :].rearrange(
                    "p (g b t) -> p g b t", g=G, b=8, t=tc_steps)

                def emit_u(t):
                    # u for step t opens the (zero-initialized) static psum
                    # banks; col-tiled z partials land on top
                    zp = [zpsS[h][t % 2] for h in range(2)]
                    for h in range(2):
                        nc.tensor.matmul(
                            zp[h][0:8, :],
                            inp_sb[:, t * bc: (t + 1) * bc],
                            win_sb[:, 512 * h: 512 * (h + 1)],
                            start=True, stop=False, skip_group_check=True,
                        )
                    return zp

                zps_cur = emit_u(0)
                for t in range(tc_steps):
                    xT, xT_n = xTs[t % 2], xTs[(t + 1) % 2]
                    xh16, xh16_n = xh16s[t % 2], xh16s[(t + 1) % 2]
                    s16, s16_n = s16s[t % 2], s16s[(t + 1) % 2]
                    zps = zps_cur
                    for h in range(2):
                        nsl = slice(512 * h, 512 * (h + 1))
                        # 4-way col-tiled: strip j takes k-tiles g=j and g=j+4,
                        # partials land at psum partitions 32j..32j+8
                        for r in range(2):
                            for j in range(4):
                                g = 4 * r + j
                                nc.tensor.matmul(
                                    zps[h][0:8, :],
                                    s16[g // Gh][:, (g % Gh) * 8: (g % Gh) * 8 + 8],
                                    w_sb[:, g * n + nsl.start: g * n + nsl.stop],
                                    start=False, stop=(r == 1 and j == 3),
                                    skip_group_check=True,
                                )
                    if t + 1 < tc_steps:
                        zps_cur = emit_u(t + 1)  # fills the PE tail gap
                    for h in range(2):
                        # strip-reduce + transpose fused on PE:
                        # zT[nloc, b] = sum_p zp16[p, nloc] * sel[p, b]
                        zp16 = work_pool.tile([128, 512], FP16, tag=f"zp{h}",
                                              name=f"zp{h}")
                        nc.scalar.copy(zp16[:, :], zps[h][:, :])
                        zTp = psum_pool.tile([128, Gh * 8], FP32,
                                             tag=f"zT{h}", name=f"zTp{h}")
                        for c in range(4):
                            nc.tensor.matmul(
                                zTp[:, 8 * c: 8 * c + 8],
                                zp16[:, 128 * c: 128 * c + 128],
                                sel_sb[:, :],
                                start=(c == 0), stop=(c == 3),
                                skip_group_check=True,
                            )
                        hT = work_pool.tile([128, Gh * 8], FP32, tag=f"hT{h}",
                                            name=f"hT{h}")
                        nc.scalar.activation(
                            hT[:, :], zTp[:, :],
                            mybir.ActivationFunctionType.Tanh)
                        # critical: next matmul operand in one add
                        nc.vector.tensor_add(
                            s16_n[h][:, :], xh16[h][:, :], hT[:, :])
                        # off the critical path: fp32 master + output
                        s = work_pool.tile([128, Gh * 8], FP32, tag=f"s{h}",
                                           name=f"s{h}")
                        s_r = s[:, :].rearrange("p (g b) -> p g b", g=Gh, b=8)
                        nc.vector.tensor_add(s[:, :], xT[h][:, :], hT[:, :])
                        nc.scalar.mul(xT_n[h][:, :], s[:, :], ALPHA)
                        nc.vector.tensor_scalar_mul(
                            xh16_n[h][:, :], s[:, :], ALPHA)
                        nc.scalar.mul(
                            obuf_r[:, Gh * h: Gh * (h + 1), :, t],
                            s_r[:, :, :], ALPHA)

                for g in range(G):
                    nc.sync.dma_start(
                        x_dram_r[:, g, :, ds(ci * tc_steps, tc_steps)],
                        obuf_r[:, g, :, :],
                    )

            with tc.For_i(0, n_chunks, 1) as i:
                chunk_body(i)

    _split_excess_waits(nc)
    return nc


def kernel(Input, W_in, W):
    """Full inputs in, full output out. Shards batch over 8 NeuronCores."""
    global LAST_EXEC_NS, _CACHED_NC
    Input = np.ascontiguousarray(np.asarray(Input, dtype=np.float32))
    W_in = np.ascontiguousarray(np.asarray(W_in, dtype=np.float32))
    W = np.ascontiguousarray(np.asarray(W, dtype=np.float32))
    Bf, n_in, t_total = Input.shape
    n = W.shape[0]
    G = n // 128
    bc = Bf // N_CORES

    tc_steps = TC if t_total % TC == 0 else max(
        d for d in range(1, min(TC, t_total) + 1) if t_total % d == 0)
    if _CACHED_NC is None:
        _CACHED_NC = _build_nc(n=n, t_total=t_total, tc_steps=tc_steps,
                               n_in=n_in, bc=bc)
    nc = _CACHED_NC

    # leak factor folded into W: matmul operand is s = x + h = 2x, so W/2
    w_r = np.ascontiguousarray(
        (ALPHA * W).reshape(G, 128, n).transpose(1, 0, 2).reshape(128, G * n)
    ).astype(np.float16)
    win16 = W_in.astype(np.float16)
    sel = np.zeros((128, 8), dtype=np.float16)
    for b_ in range(8):
        sel[b_, b_] = 1.0
    in_maps = []
    for c in range(N_CORES):
        inpT = np.ascontiguousarray(
            Input[c * bc:(c + 1) * bc].transpose(1, 2, 0)).astype(np.float16)
        in_maps.append({"w": w_r, "win": win16, "inpT": inpT, "sel": sel})

    trace = bool(int(os.environ.get("ESN_TRACE", "0")))
    res = run_bass_kernel_spmd(
        nc, in_maps, core_ids=list(range(N_CORES)), trace=trace)
    LAST_EXEC_NS = res.exec_time_ns

    out = np.concatenate([res.results[c]["xout"] for c in range(N_CORES)],
                         axis=0)
    return np.ascontiguousarray(out.astype(np.float32))


# revision 20
# speedup vs baseline: 1.0869x; 1.0006x over previous
"""Trainium2 Bass kernel for the ESN (echo state network) forward scan.

  x_{t+1} = (1-a) x_t + a tanh(u_t + x_t @ W),  a = 0.5
  U = einsum('bit,in->tbn', Input, W_in);  out X[b,n,t] = x_{t+1}[b,n]

Sharding: data-parallel over batch (B=64 -> 8 cores x 8 batches).
W, W_in replicated; no collectives. Each core runs the full T=2000 scan
for its 8 batches and writes its [8, 1024, 2000] output slice.

Per-core per-step data flow (all on-chip, only X streamed out):
  PE:  z[8,1024](PSUM) = xT.T @ W (8 k-tile matmuls x 2 psum banks)
                        + inp_t.T @ W_in (u folded in as a K=16 matmul)
  DVE: z -> zT [128, 64] (32x32 block transposes, strided APs)
  ACT: hT = tanh(zT)
  DVE: s = xT + hT; xT' = 0.5 s; obuf[:, :, t] = 0.5 s   (x_{t+1})
Output chunks of Tc steps buffered in SBUF, DMA'd as [128, Tc]-contiguous
blocks into X[b, 128g:128g+128, t0:t0+Tc].
"""

import copy
import math
import os
import numpy as np

import concourse.bass as bass
import concourse.mybir as mybir
import concourse.tile as tile
from concourse.bass import ds
from concourse.bass_utils import run_bass_kernel_spmd

FP32 = mybir.dt.float32
FP16 = mybir.dt.float16

ALPHA = 0.5
N_CORES = 8
B, N_IN, T, N = 64, 16, 2000, 1024
TC = 100  # steps buffered per output chunk

LAST_EXEC_NS = None
_CACHED_NC = None


def _split_excess_waits(nc, limit=1):
    """The walrus build in this container rejects instructions carrying more
    than one sem wait; hoist extra waits onto same-engine NoOps."""
    import bass_rust
    for f in nc.m.functions:
        for bb in f.blocks:
            new_insts = []
            for ins in bb.instructions:
                si = ins.sync_info
                if si is not None and si.on_wait and len(si.on_wait) > limit:
                    waits = list(si.on_wait)
                    head, tail = waits[:-limit], waits[-limit:]
                    for j, w in enumerate(head):
                        c = bass_rust.InstNoOp(name=f"{ins.name}-w{j}")
                        c.engine = ins.engine
                        c.sync_info = mybir.SyncInfo(on_wait=[w], on_update=[])
                        new_insts.append(c)
                    si.on_wait = tail
                new_insts.append(ins)
            bb.instructions = new_insts
    return nc


def _build_nc(n=N, t_total=T, tc_steps=TC, n_in=N_IN, bc=B // N_CORES):
    G = n // 128
    n_chunks = t_total // tc_steps
    NB = (n + 511) // 512
    nb_sizes = [min(512, n - 512 * i) for i in range(NB)]

    assert NB == 2 and G % 2 == 0
    Gh = G // 2  # g-tiles per n-half

    nc = bass.Bass()
    sel_dram = nc.dram_tensor("sel", [128, 8], FP16, kind="ExternalInput")
    w_dram = nc.dram_tensor("w", [128, G * n], FP16, kind="ExternalInput")
    win_dram = nc.dram_tensor("win", [n_in, n], FP16, kind="ExternalInput")
    inpT_dram = nc.dram_tensor("inpT", [n_in, t_total, bc], FP16,
                               kind="ExternalInput")
    x_dram = nc.dram_tensor("xout", [bc, n, t_total], FP32,
                            kind="ExternalOutput")
    x_dram_r = x_dram.rearrange("b (g p) t -> p g b t", p=128)

    with tile.TileContext(nc) as tc:
        with (
            tc.tile_pool(name="const", bufs=1) as const_pool,
            tc.tile_pool(name="state", bufs=1) as state_pool,
            tc.tile_pool(name="work", bufs=3) as work_pool,
            tc.tile_pool(name="obuf", bufs=2) as obuf_pool,
            tc.tile_pool(name="inp", bufs=2) as inp_pool,
            tc.tile_pool(name="psum", bufs=2, space="PSUM") as psum_pool,
            tc.tile_pool(name="psumS", bufs=1, space="PSUM") as psum_static,
        ):
            w_sb = const_pool.tile([128, G * n], FP16)
            nc.sync.dma_start(w_sb[:, :], w_dram[:, :])
            win_sb = const_pool.tile([n_in, n], FP16)
            nc.sync.dma_start(win_sb[:, :], win_dram[:, :])
            sel_sb = const_pool.tile([128, 8], FP16)
            nc.sync.dma_start(sel_sb[:, :], sel_dram[:, :])
            zero16 = const_pool.tile([128, 512], FP16)
            nc.vector.memset(zero16[:, :], 0.0)

            # 4 static psum banks for the col-tiled z partials (ping-pong per
            # step); zero-filled once so never-written partition rows stay
            # finite (sel rows are 0 there, and PE treats 0*garbage as NaN
            # if garbage were NaN)
            zpsS = [[psum_static.tile([128, 512], FP32, name=f"zps_{h_}_{b_}")
                     for b_ in range(2)] for h_ in range(2)]
            for h_ in range(2):
                for b_ in range(2):
                    nc.tensor.matmul(
                        zpsS[h_][b_][:, :], zero16[:, 0:128], zero16[:, :],
                        start=True, stop=True, skip_group_check=True)

            # State kept three ways, split per n-half for fine dep granularity:
            #   s16[.]  fp16 unscaled sum s_t = x_t + h_t -> matmul operand
            #           (the 0.5 leak is folded into W host-side)
            #   xT[.]   fp32 master of x_{t+1} = 0.5 s_t (exact output)
            #   xh16[.] fp16 of x_t, feeds the critical add s16 = xh16 + h
            s16s = [[state_pool.tile([128, Gh * 8], FP16, name=f"s16_{b_}_{h_}")
                     for h_ in range(2)] for b_ in range(2)]
            xTs = [[state_pool.tile([128, Gh * 8], FP32, name=f"xT{b_}_{h_}")
                    for h_ in range(2)] for b_ in range(2)]
            xh16s = [[state_pool.tile([128, Gh * 8], FP16,
                                      name=f"xh16_{b_}_{h_}")
                      for h_ in range(2)] for b_ in range(2)]
            for b_ in range(2):
                for h_ in range(2):
                    nc.vector.memset(s16s[b_][h_][:, :], 0.0)
                    nc.vector.memset(xTs[b_][h_][:, :], 0.0)
                    nc.vector.memset(xh16s[b_][h_][:, :], 0.0)

            def chunk_body(ci):
                inp_sb = inp_pool.tile([n_in, tc_steps * bc], FP16)
                nc.sync.dma_start(
                    inp_sb[:, :], inpT_dram[:, ds(ci * tc_steps, tc_steps), :])
                obuf = obuf_pool.tile([128, G * 8 * tc_steps], FP32)
                obuf_r = obuf[:, :].rearrange(
                    "p (g b t) -> p g b t", g=G, b=8, t=tc_steps)

                def emit_u(t):
                    # u for step t opens the (zero-initialized) static psum
                    # banks; col-tiled z partials land on top
                    zp = [zpsS[h][t % 2] for h in range(2)]
                    for h in range(2):
                        nc.tensor.matmul(
                            zp[h][0:8, :],
                            inp_sb[:, t * bc: (t + 1) * bc],
                            win_sb[:, 512 * h: 512 * (h + 1)],
                            start=True, stop=False, skip_group_check=True,
                        )
                    return zp

                zps_cur = emit_u(0)
                for t in range(tc_steps):
                    xT, xT_n = xTs[t % 2], xTs[(t + 1) % 2]
                    xh16, xh16_n = xh16s[t % 2], xh16s[(t + 1) % 2]
                    s16, s16_n = s16s[t % 2], s16s[(t + 1) % 2]
                    zps = zps_cur
                    for h in range(2):
                        nsl = slice(512 * h, 512 * (h + 1))
                        for g in range(G):
                            nc.tensor.matmul(
                                zps[h][0:8, :],
                                s16[g // Gh][:, (g % Gh) * 8: (g % Gh) * 8 + 8],
                                w_sb[:, g * n + nsl.start: g * n + nsl.stop],
                                start=False, stop=(g == G - 1),
                                skip_group_check=True,
                            )
                    if t + 1 < tc_steps:
                        zps_cur = emit_u(t + 1)  # fills the PE tail gap
                    for h in range(2):
                        # strip-reduce + transpose fused on PE:
                        # zT[nloc, b] = sum_p zp16[p, nloc] * sel[p, b]
                        zp16 = work_pool.tile([128, 512], FP16, tag=f"zp{h}",
                                              name=f"zp{h}")
                        nc.scalar.copy(zp16[:, :], zps[h][:, :])
                        zTp = psum_pool.tile([128, Gh * 8], FP32,
                                             tag=f"zT{h}", name=f"zTp{h}")
                        for c in range(4):
                            nc.tensor.matmul(
                                zTp[:, 8 * c: 8 * c + 8],
                                zp16[:, 128 * c: 128 * c + 128],
                                sel_sb[:, :],
                                start=(c == 0), stop=(c == 3),
                                skip_group_check=True,
                            )
                        hT = work_pool.tile([128, Gh * 8], FP32, tag=f"hT{h}",
                                            name=f"hT{h}")
                        nc.scalar.activation(
                            hT[:, :], zTp[:, :],
                            mybir.ActivationFunctionType.Tanh)
                        # critical: next matmul operand in one add
                        nc.vector.tensor_add(
                            s16_n[h][:, :], xh16[h][:, :], hT[:, :])
                        # off the critical path: fp32 master + output
                        s = work_pool.tile([128, Gh * 8], FP32, tag=f"s{h}",
                                           name=f"s{h}")
                        s_r = s[:, :].rearrange("p (g b) -> p g b", g=Gh, b=8)
                        nc.vector.tensor_add(s[:, :], xT[h][:, :], hT[:, :])
                        nc.scalar.mul(xT_n[h][:, :], s[:, :], ALPHA)
                        nc.vector.tensor_scalar_mul(
                            xh16_n[h][:, :], s[:, :], ALPHA)
                        nc.scalar.mul(
                            obuf_r[:, Gh * h: Gh * (h + 1), :, t],
                            s_r[:, :, :], ALPHA)

                for g in range(G):
                    nc.sync.dma_start(
                        x_dram_r[:, g, :, ds(ci * tc_steps, tc_steps)],
                        obuf_r[:, g, :, :],
                    )

            with tc.For_i(0, n_chunks, 1) as i:
                chunk_body(i)

    _split_excess_waits(nc)
    return nc


def kernel(Input, W_in, W):
    """Full inputs in, full output out. Shards batch over 8 NeuronCores."""
    global LAST_EXEC_NS, _CACHED_NC
    Input = np.ascontiguousarray(np.asarray(Input, dtype=np.float32))
    W_in = np.ascontiguousarray(np.asarray(W_in, dtype=np.float32))
    W = np.ascontiguousarray(np.asarray(W, dtype=np.float32))
    Bf, n_in, t_total = Input.shape
    n = W.shape[0]
    G = n // 128
    bc = Bf // N_CORES

    tc_steps = TC if t_total % TC == 0 else max(
        d for d in range(1, min(TC, t_total) + 1) if t_total % d == 0)
    if _CACHED_NC is None:
        _CACHED_NC = _build_nc(n=n, t_total=t_total, tc_steps=tc_steps,
                               n_in=n_in, bc=bc)
    nc = _CACHED_NC

    # leak factor folded into W: matmul operand is s = x + h = 2x, so W/2
    w_r = np.ascontiguousarray(
        (ALPHA * W).reshape(G, 128, n).transpose(1, 0, 2).reshape(128, G * n)
    ).astype(np.float16)
    win16 = W_in.astype(np.float16)
    sel = np.zeros((128, 8), dtype=np.float16)
    for b_ in range(8):
        sel[b_, b_] = 1.0
    in_maps = []
    for c in range(N_CORES):
        inpT = np.ascontiguousarray(
            Input[c * bc:(c + 1) * bc].transpose(1, 2, 0)).astype(np.float16)
        in_maps.append({"w": w_r, "win": win16, "inpT": inpT, "sel": sel})

    trace = bool(int(os.environ.get("ESN_TRACE", "0")))
    res = run_bass_kernel_spmd(
        nc, in_maps, core_ids=list(range(N_CORES)), trace=trace)
    LAST_EXEC_NS = res.exec_time_ns

    out = np.concatenate([res.results[c]["xout"] for c in range(N_CORES)],
                         axis=0)
    return np.ascontiguousarray(out.astype(np.float32))
